# revision 31
# baseline (speedup 1.0000x reference)
"""LeNet-style CNN (conv5x5+avgpool2+sigmoid x2, then 3 FC layers) on 8 trn2
NeuronCores, pure data parallel over the batch.

v2 key ideas (on top of the fused-conv v1):
- conv+pool fused to a 6x6 stride-2 conv (pooling is linear), expressed as
  accumulating matmuls with Toeplitz weights over strided SBUF views.
- Layer-1 K=120 merging: x is laid out host-side as
  [(row mod 4, width) -> partitions 0-55 (m=0,1) and 64-119 (m=2,3), zeros in
  56-63], so the three K=56 contributions per output row merge into TWO
  K=120 full-array accumulating matmuls (adjacent kernel-row pairs read the
  same 4-row group; the unused half of each stationary is zero): 72 serial
  matmul slots become 48. (True A/B row-strip pairing dies on HW: an
  accumulation group must keep one tile_position across its matmuls.)
- h1 is stored [120 parts = (pj,o), block pi, batch], so one merged
  bias+sigmoid ACT per L1 pair covers [120, 2, 1024] (fewer ScalarE
  fixed overheads; ScalarE is the co-bottleneck).
- L1 pairs and L2 qi-tiles are interleaved (p0 p1 p2 q0 p3 q1 p4 q2 p5 q3)
  to balance PE (L2-heavy) vs ScalarE (L1-heavy) load.
- FC3 runs as a moving-batch matmul ([85,10] stationary, batch moving) so
  the output is [10, 1024]: 2 matmuls + 2 vector copies instead of 8
  stationary-activation tiles, with per-half output DMA.
- Weights packed into 4 DMAs total; x in 4 group-chunk DMAs, all on the
  sync HWDGE ring, ordered by first use.
"""

import numpy as np
import ml_dtypes
import concourse.bacc as bacc
import concourse.mybir as mybir
import concourse.tile as tile
from concourse.vector_clock import ScopedClock
from concourse.bass_utils import run_bass_kernel_spmd

F32 = mybir.dt.float32
F32R = mybir.dt.float32r
BF16 = mybir.dt.bfloat16
FP8 = mybir.dt.float8e4
SIG = mybir.ActivationFunctionType.Sigmoid

N_CORES = 8
B_FULL = 8192
NB = B_FULL // N_CORES  # 1024 images per core
HB = 512  # matmul moving-dim tile (one PSUM bank of fp32)
N_WARM = 8


class SlimTailTileContext(tile.TileContext):
    """Tile's standard teardown emits drain + all-engine barrier + semaphore
    clears + another barrier (~10us on HW). This NEFF executes exactly once
    per load, so the semaphore-reset choreography is dead weight: keep the
    data-completeness drain, do the allocator bookkeeping host-side only."""

    def _drain_and_barrier(self, tick_clock, wait_clock):
        drain_inst = self.nc.sync.drain()
        wait_clock.add_sem_waits(
            drain_inst.ins, ScopedClock({None: tick_clock.global_clock})
        )
        popped = self.nc._tile_sem_poison_stack.pop()
        assert popped is self._sem_poison
        sems = list(self.sems.allocated().values())
        sem_nums = [sm.num for sm in sems]
        self.nc._state.prepend_free_semaphores(sem_nums)
        for poison_set in self.nc._tile_sem_poison_stack:
            poison_set.update(sem_nums)


def _fuse_pool(W):
    """conv(W, stride 1) + 2x2 mean-pool == conv(Wf, stride 2), Wf 6x6."""
    O, C, _, _ = W.shape
    Wf = np.zeros((O, C, 6, 6), np.float32)
    for u in (0, 1):
        for v in (0, 1):
            Wf[:, :, u : u + 5, v : v + 5] += W
    return Wf * 0.25


def _host_weights(W1, b1, W2, b2, L1, Lb1, L2, Lb2, L3, Lb3):
    W1f = _fuse_pool(np.asarray(W1, np.float32))  # [10,1,6,6]
    W2f = _fuse_pool(np.asarray(W2, np.float32))  # [20,10,6,6]

    # Layer 1 Toeplitz: S_k[(m',w), (pj,o)] = W1f[o, 0, 2k+m', w-2pj],
    # merged into 4 zero-padded [120,120] stationaries (partition halves
    # 0-55 / 64-119 are the two kernel-row-pair positions of a 4-row group):
    # blk0 = [S_0; 0; S_1] (pi even, first group), blk1 = [S_2; 0; 0]
    # blk2 = [0; 0; S_0] (pi odd, first group),   blk3 = [S_1; 0; S_2]
    S = np.zeros((3, 56, 120), np.float32)
    for k in range(3):
        for mp in range(2):
            e = 2 * k + mp
            for pj in range(12):
                for f in range(6):
                    w = 2 * pj + f
                    S[k, mp * 28 + w, pj * 10 : pj * 10 + 10] = W1f[:, 0, e, f]
    t1 = np.zeros((120, 4, 128), np.float32)
    t1[0:56, 0, 0:120] = S[0]
    t1[64:120, 0, 0:120] = S[1]
    t1[0:56, 1, 0:120] = S[2]
    t1[64:120, 2, 0:120] = S[0]
    t1[0:56, 3, 0:120] = S[1]
    t1[64:120, 3, 0:120] = S[2]

    # Layer 2 Toeplitz, fp8 DoubleRow layout: block j2 = 2k+par so moving
    # h1 block pairs (2(qi+k), 2(qi+k)+1) pair with stationary blocks
    # (2k, 2k+1); free dim padded 80 -> 96 for alignment.
    t2 = np.zeros((120, 6, 96), np.float32)
    for par in range(2):
        for k in range(3):
            e = 2 * k + par
            j2 = 2 * k + par
            for qj in range(4):
                for f in range(6):
                    pj = 2 * qj + f
                    for c in range(10):
                        t2[pj * 10 + c, j2, qj * 20 : qj * 20 + 20] = W2f[:, c, e, f]

    # biases + final linear packed into one tiny [120, 14] fp32 DMA:
    # col 0: bias1 (120), col 1: bias2 (80), col 2: lb1 (120), col 3: lb2 (84),
    # cols 4:14: [L3; Lb3] (85 rows)
    bt = np.zeros((120, 14), np.float32)
    bt[:, 0] = np.tile(np.asarray(b1, np.float32).reshape(10), 12)
    bt[0:80, 1] = np.tile(np.asarray(b2, np.float32).reshape(20), 4)
    bt[:, 2] = np.asarray(Lb1, np.float32).reshape(120)
    bt[0:84, 3] = np.asarray(Lb2, np.float32).reshape(84)
    bt[0:84, 4:14] = np.asarray(L3, np.float32)
    bt[84, 4:14] = np.asarray(Lb3, np.float32).reshape(10)

    # FC1 permuted for the [(qj,oc) partitions, (qi,b) free] input layout,
    # packed with the FC2 matrix into one [120, 564] fp32 DMA.
    wfc = np.zeros((120, 564), np.float32)
    L1a = np.asarray(L1, np.float32)
    for qi in range(4):
        for qj in range(4):
            for oc in range(20):
                wfc[qj * 20 + oc, qi * 120 : (qi + 1) * 120] = L1a[
                    oc * 16 + qi * 4 + qj
                ]
    wfc[:, 480:564] = np.asarray(L2, np.float32)  # [120, 84]

    bf = ml_dtypes.bfloat16
    return {
        "t1": np.ascontiguousarray(t1, dtype=ml_dtypes.float8_e4m3),
        "t2": np.ascontiguousarray(t2, dtype=ml_dtypes.float8_e4m3),
        "bt": np.ascontiguousarray(bt),
        "wfc": np.ascontiguousarray(wfc),
    }


def _build_nc():
    nc = bacc.Bacc()
    xp = nc.dram_tensor("xp", [120, 7, NB], FP8, kind="ExternalInput")
    t2 = nc.dram_tensor("t2", [120, 6, 96], FP8, kind="ExternalInput")
    # F32R so the verifier accepts these as fp32r-matmul inputs (same bits)
    t1 = nc.dram_tensor("t1", [120, 4, 128], FP8, kind="ExternalInput")
    bt = nc.dram_tensor("bt", [120, 14], F32R, kind="ExternalInput")
    wfc = nc.dram_tensor("wfc", [120, 564], F32R, kind="ExternalInput")
    y = nc.dram_tensor("y", [10, NB], F32, kind="ExternalOutput")

    with SlimTailTileContext(nc) as tc:
        with (
            tc.tile_pool(name="w", bufs=1) as wp,
            tc.tile_pool(name="act", bufs=1) as ap,
            tc.tile_pool(name="ps", bufs=4, space="PSUM") as psp,
        ):
            # --- warm-up scaffolding: PE HAM clock-gate starts at 1.2 GHz and
            # reaches 2.4 GHz only after ~3.4us of sustained matmul activity;
            # dependency-free dummy matmuls during the input DMA phase warm it.
            # A dummy sigmoid also pulls the ACT table load off the critical
            # path.
            bts = wp.tile([120, 14], F32R, tag="bt")
            warm = wp.tile([128, 640], BF16, tag="warm")
            nc.vector.memset(warm[:, :], 0.0)
            warmf = wp.tile([128, 16], F32, tag="warmf")
            nc.vector.memset(warmf[:, :], 0.0)
            nc.scalar.activation(warmf[:, 8:16], warmf[:, 0:8], SIG)
            for _ in range(N_WARM):
                wps = psp.tile([128, HB], F32, tag="ps")
                nc.tensor.matmul(
                    wps[:], warm[:, :128], warm[:, 128:640], start=True, stop=True
                )

            # --- SBUF residents ---
            xs = ap.tile([120, 7, NB], FP8, tag="xp")
            t2s = wp.tile([120, 6, 96], FP8, tag="t2")
            t1s = wp.tile([120, 4, 128], FP8, tag="t1")
            wfcs = wp.tile([120, 564], F32R, tag="wfc")

            b1s = bts[:, 0:1].bitcast(F32)
            b2s = bts[0:80, 1:2].bitcast(F32)
            lb1s = bts[:, 2:3].bitcast(F32)
            lb2s = bts[0:84, 3:4].bitcast(F32)
            l3s = bts[0:85, 4:14]
            l1s = wfcs[0:80, 0:480]
            l2s = wfcs[:, 480:564]

            h1 = ap.tile([120, 12, NB], FP8, tag="h1")  # [.., block pi * NB + b]
            h2 = ap.tile([80, 4 * NB], F32R, tag="h2")  # [(qj,oc), qi*NB+b]
            h3 = ap.tile([120, NB], F32R, tag="h3")
            h4 = ap.tile([85, NB], F32R, tag="h4")  # row 84 == 1.0 (FC3 bias)
            ys = ap.tile([10, NB], F32, tag="ys")

            # Row 84 must be 1.0 (FC3 bias row); FC2's activation later
            # overwrites rows 0..83.
            nc.gpsimd.memset(h4[:, :].bitcast(F32), 1.0)

            # --- input DMAs, sync HWDGE ring, ordered by first use ---
            def xchunk(g0, g1):
                nc.sync.dma_start(xs[:, g0:g1, :], xp[:, g0:g1, :])

            nc.sync.dma_start(t1s[:], t1[:])
            xchunk(0, 2)
            xchunk(2, 4)
            nc.sync.dma_start(bts[:], bt[:])
            nc.sync.dma_start(t2s[:], t2[:])
            xchunk(4, 6)
            xchunk(6, 7)
            nc.sync.dma_start(wfcs[:], wfc[:])

            # --- layer 1, output row pi: two K=120 full-array accumulating
            # matmuls per (pi, half) over x groups g=pi//2 and pi//2+1;
            # stationaries zero-padded so each contributes only its valid
            # kernel-row pairs. (An accumulation group must keep ONE
            # tile_position across its matmuls on HW, so K=56 A/B row-strip
            # pairing is not an option.)
            # fp8 DoubleRow: both K=120 group-contributions ride one matmul
            # (virtual 256-row array; stationary [120, 2, 128] middle dim =
            # group step, moving [120, 2, 512] = x groups g, g+1).
            def l1_pi(pi, split=False):
                ps = psp.tile([128, 1024], F32, tag="ps")
                g = pi // 2
                p = pi % 2
                for h in range(2):
                    b0 = h * HB
                    nc.tensor.matmul(
                        ps[:, b0 : b0 + HB],
                        t1s[:, 2 * p : 2 * p + 2, :],
                        xs[:, g : g + 2, b0 : b0 + HB],
                        start=True,
                        stop=True,
                        perf_mode=mybir.MatmulPerfMode.DoubleRow,
                    )
                    if split:
                        nc.scalar.activation(
                            h1[:, pi, b0 : b0 + HB], ps[0:120, b0 : b0 + HB],
                            SIG, bias=b1s,
                        )
                if not split:
                    nc.scalar.activation(
                        h1[:, pi, :], ps[0:120, :], SIG, bias=b1s
                    )

            # --- layer 2 output-row qi: 2 halves x 6 accumulating matmuls,
            # one merged ACT per qi.
            l2_ps = {}

            def l2_qi_mms(qi, ks):
                if qi not in l2_ps:
                    l2_ps[qi] = psp.tile([96, 1024], F32, tag="ps", name=f"psq{qi}")
                ps = l2_ps[qi]
                for h in range(2):
                    b0 = h * HB
                    for k in ks:
                        nc.tensor.matmul(
                            ps[:, h * HB : h * HB + HB],
                            t2s[:, 2 * k : 2 * k + 2, :],
                            h1[:, 2 * (qi + k) : 2 * (qi + k) + 2, b0 : b0 + HB],
                            start=(k == 0),
                            stop=(k == 2),
                            perf_mode=mybir.MatmulPerfMode.DoubleRow,
                        )

            def l2_qi(qi):
                ps = l2_ps[qi]
                if qi == 3:
                    for h in range(2):
                        b0 = h * HB
                        nc.scalar.activation(
                            h2[:, qi * NB + b0 : qi * NB + b0 + HB],
                            ps[0:80, b0 : b0 + HB],
                            SIG,
                            bias=b2s,
                        )
                else:
                    nc.scalar.activation(
                        h2[:, qi * NB : (qi + 1) * NB], ps[0:80, :], SIG, bias=b2s
                    )

            # interleave at k-granularity: q_i's step k needs p_(2i+2k+1);
            # emitting (k0,k1) after p_(2i+3) and (k2 + ACT) after p_(2i+5)
            # lets the PE fill h1-wait bubbles with ready work.
            l1_pi(0)
            l1_pi(1)
            l1_pi(2)
            l1_pi(3)
            l2_qi_mms(0, (0, 1))
            l1_pi(4)
            l1_pi(5)
            l2_qi_mms(0, (2,))
            l2_qi(0)
            l2_qi_mms(1, (0, 1))
            l1_pi(6)
            l1_pi(7)
            l2_qi_mms(1, (2,))
            l2_qi(1)
            l2_qi_mms(2, (0, 1))
            l1_pi(8)
            l1_pi(9)
            l2_qi_mms(2, (2,))
            l2_qi(2)
            l2_qi_mms(3, (0, 1))
            l1_pi(10)
            l1_pi(11)
            l2_qi_mms(3, (2,))
            l2_qi(3)

            # --- FC1/FC2/FC3 pipeline, per-half psum tiles so the two
            # batch halves overlap (a shared tile serializes h1 matmuls
            # behind h0's ACT read). FC3 is batch-moving ([85,10] stationary)
            # with per-half copy + output DMA so the tail drains early.
            QB = 256
            ps1 = [psp.tile([120, QB], F32, tag="ps", name=f"ps1{q}") for q in range(4)]
            ps2 = [psp.tile([84, QB], F32, tag="ps", name=f"ps2{q}") for q in range(4)]
            ps3 = [psp.tile([10, QB], F32, tag="ps", name=f"ps3{q}") for q in range(4)]
            for q in range(4):
                b0 = q * QB
                for qi in range(4):
                    nc.tensor.matmul(
                        ps1[q][:, :],
                        l1s[:, qi * 120 : (qi + 1) * 120],
                        h2[:, qi * NB + b0 : qi * NB + b0 + QB],
                        start=(qi == 0),
                        stop=(qi == 3),
                    )
                nc.scalar.activation(
                    h3[:, b0 : b0 + QB], ps1[q][:, :], SIG, bias=lb1s
                )
            for q in range(4):
                b0 = q * QB
                nc.tensor.matmul(
                    ps2[q][:, :], l2s, h3[:, b0 : b0 + QB], start=True, stop=True
                )
                nc.scalar.activation(
                    h4[0:84, b0 : b0 + QB], ps2[q][:, :], SIG, bias=lb2s
                )
            for q in range(4):
                b0 = q * QB
                nc.tensor.matmul(
                    ps3[q][:, :], l3s, h4[:, b0 : b0 + QB], start=True, stop=True
                )
                nc.vector.tensor_copy(ys[:, b0 : b0 + QB], ps3[q][:, :])
            nc.sync.dma_start(y[:, :], ys[:, :])
    nc.compile()
    return nc


_NC_CACHE = None


def _get_nc():
    global _NC_CACHE
    if _NC_CACHE is None:
        _NC_CACHE = _build_nc()
    return _NC_CACHE


def _make_in_maps(x, W1, b1, W2, b2, L1, Lb1, L2, Lb2, L3, Lb3):
    wmap = _host_weights(W1, b1, W2, b2, L1, Lb1, L2, Lb2, L3, Lb3)
    x = np.asarray(x, dtype=np.float32)
    bf = ml_dtypes.bfloat16
    in_maps = []
    for c in range(N_CORES):
        xc = x[c * NB : (c + 1) * NB, 0]  # [NB, 28, 28]
        # rows r = 4g + m; partitions: m in {0,1} -> 0:56, m in {2,3} -> 64:120
        v = xc.reshape(NB, 7, 4, 28).transpose(2, 3, 1, 0).reshape(112, 7, NB)
        xpc = np.zeros((120, 7, NB), dtype=ml_dtypes.float8_e4m3)
        xpc[0:56] = v[0:56]
        xpc[64:120] = v[56:112]
        m = {"xp": xpc}
        m.update(wmap)
        in_maps.append(m)
    return in_maps


def _run(trace=False, **inputs):
    global _NC_CACHE
    nc = _get_nc()
    in_maps = _make_in_maps(**inputs)
    res = run_bass_kernel_spmd(nc, in_maps, list(range(N_CORES)), trace=trace)
    # the slim teardown leaves semaphores dirty; force a fresh NEFF if
    # kernel() is ever called again in this process
    _NC_CACHE = None
    outs = []
    for i in range(N_CORES):
        yc = res.results[i]["y"]  # [10, NB]
        outs.append(yc.T)
    out = np.ascontiguousarray(np.concatenate(outs, axis=0))
    return out, res


def kernel(**inputs):
    out, _ = _run(trace=False, **inputs)
    return out
